# revision 4
# baseline (speedup 1.0000x reference)
"""DeconvCapsuleLayer Trainium2 kernel, v2.

Data-parallel over batch (B=8 -> 1 image per NeuronCore).
Device kernel per core (same structure as v1, fp16 I/O):
  - deconv as 4 sub-pixel phases; each phase = 4 taps of K=32 fp16 matmuls
    accumulated in fp32 PSUM (W stationary, out = [64(oc,oa), pixels]).
  - PE transpose to pixel-major [pixels, (ic,oc,oa)].
  - dynamic routing (3 iters) on DVE/ACT in fp32, output stored fp16.
Host: fp16 inputs/outputs to halve the axon link traffic; the jitted
shard_map dispatch is built once and cached (the stock
run_bass_kernel_spmd path re-traces and re-lowers on every call).
"""

import os
import sys
from contextlib import ExitStack

import numpy as np

for _p in ("/opt/trn_rl_repo", os.path.expanduser("~/.axon_site/_ro/trn_rl_repo")):
    if os.path.isdir(_p) and _p not in sys.path:
        sys.path.insert(0, _p)

import concourse.bass as bass
import concourse.bacc as bacc
import concourse.tile as tile
from concourse import mybir
from concourse.bass_utils import run_bass_kernel_spmd  # noqa: F401 (fallback)

F32 = mybir.dt.float32
F16 = mybir.dt.float16
I8 = mybir.dt.int8
OSCALE = 124.0  # int8 output fixed-point scale (|act| < 1; margin for rounding)
AX = mybir.AxisListType
OP = mybir.AluOpType
AF = mybir.ActivationFunctionType

B, H, Wd, IC, IA = 8, 56, 56, 8, 32
OC, OA = 4, 16
PH, PW = 58, 58  # padded input spatial
NPIX = 56 * 56   # pixels per phase image
# tap tables: KH[parity] = kernel taps, DH[parity] = input shifts
KH = {0: [1, 3], 1: [0, 2]}
DH = {0: [0, -1], 1: [1, 0]}

_CACHE = {}


def _squash_tiles(nc, pool, t_ap, out_ap, tag, scale=None):
    """out = t * sqrt(nsq)/(1+nsq) [* scale], nsq = sum_oa t^2  (t: [112, 64])."""
    sq = pool.tile([112, 64], F32, tag="mid")
    nc.vector.tensor_mul(sq[:], t_ap, t_ap)
    nsq = pool.tile([112, 4], F32, tag="sml")
    nc.vector.tensor_reduce(
        nsq[:], sq[:].rearrange("p (oc oa) -> p oc oa", oc=4), axis=AX.X, op=OP.add
    )
    s = pool.tile([112, 4], F32, tag="sml")
    nc.scalar.sqrt(s[:], nsq[:])
    u = pool.tile([112, 4], F32, tag="sml")
    nc.vector.tensor_scalar_add(u[:], nsq[:], 1.0)
    rc = pool.tile([112, 4], F32, tag="sml")
    nc.vector.reciprocal(rc[:], u[:])
    f = pool.tile([112, 4], F32, tag="sml")
    if scale is None:
        nc.vector.tensor_mul(f[:], s[:], rc[:])
    else:
        nc.vector.scalar_tensor_tensor(
            f[:], s[:], float(scale), rc[:], op0=OP.mult, op1=OP.mult
        )
    f_bc = f[:].unsqueeze(2).broadcast_to([112, 4, 16])
    t3 = t_ap.rearrange("p (oc oa) -> p oc oa", oc=4)
    nc.vector.tensor_mul(out_ap.rearrange("p (oc oa) -> p oc oa", oc=4), t3, f_bc)


def _build_nc():
    if "nc" in _CACHE:
        return _CACHE["nc"]
    nc = bacc.Bacc("TRN2", target_bir_lowering=False, debug=False)
    # raw pixel-major input [h*w, ic*ia]; transposed to capsule-major on device
    x_d = nc.dram_tensor("x", [NPIX, IC * IA], F16, kind="ExternalInput")
    wt_d = nc.dram_tensor("wt", [128, 1024], F16, kind="ExternalInput")
    cst_d = nc.dram_tensor("cst", [128, 128], F32, kind="ExternalInput")
    out_d = nc.dram_tensor("out", [4, NPIX, 64], I8, kind="ExternalOutput")

    with tile.TileContext(nc) as tc, ExitStack() as ctx:
        cpool = ctx.enter_context(tc.tile_pool(name="const", bufs=1))
        xwpool = ctx.enter_context(tc.tile_pool(name="xw", bufs=2))
        wt_sb = cpool.tile([128, 1024], F16, tag="wt")
        nc.sync.dma_start(wt_sb[:], wt_d.ap())
        cst_sb = cpool.tile([128, 128], F32, tag="cst")
        nc.sync.dma_start(cst_sb[:], cst_d.ap())
        bias_ap = cst_sb[0:112, 0:64]
        ident = cst_sb[0:64, 64:128]

        vpool = ctx.enter_context(tc.tile_pool(name="votes", bufs=2))
        pmpool = ctx.enter_context(tc.tile_pool(name="pm", bufs=2))
        pspool = ctx.enter_context(tc.tile_pool(name="ps", bufs=2, space="PSUM"))
        tppool = ctx.enter_context(tc.tile_pool(name="tp", bufs=2, space="PSUM"))
        rt = ctx.enter_context(tc.tile_pool(name="rt", bufs=10))
        opool = ctx.enter_context(tc.tile_pool(name="outp", bufs=3))

        # On-device layout transform: XBAR DMA transpose [3136, 256] ->
        # two [128, 3136] halves written into zero-padded [128, 58, 58]
        # capsule-major images (partitions = (ic_local, ia), ic-major).
        # matmul operands must share base partition in {0, 32, 64}; the 4th
        # ic block (partitions 96:128) is relocated to its own base-0 tile.
        xtiles = {}
        for hi in range(2):
            xt = cpool.tile([128, PH * PW], F16, tag=f"xsb{hi}")
            xt_v = xt[:].rearrange("k (h w) -> k h w", h=PH, w=PW)
            nc.vector.memset(xt_v[:, 0:1, :], 0.0)
            nc.vector.memset(xt_v[:, 57:58, :], 0.0)
            nc.vector.memset(xt_v[:, 1:57, 0:1], 0.0)
            nc.vector.memset(xt_v[:, 1:57, 57:58], 0.0)
            xraw = cpool.tile([128, NPIX], F16, tag=f"xraw{hi}")
            nc.sync.dma_start_transpose(
                xraw[:], x_d.ap()[:, hi * 128 : (hi + 1) * 128]
            )
            nc.scalar.copy(
                xt_v[:, 1:57, 1:57],
                xraw[:].rearrange("k (h w) -> k h w", h=56, w=56),
            )
            xc = cpool.tile([32, PH * PW], F16, tag=f"xc{hi}")
            nc.sync.dma_start(xc[:], xt[96:128, :])
            xc_v = xc[:].rearrange("k (h w) -> k h w", h=PH, w=PW)
            for loc in range(3):
                xtiles[hi * 4 + loc] = (xt_v, loc * 32)
            xtiles[hi * 4 + 3] = (xc_v, 0)

        for p in range(4):
            ph, pw = p >> 1, p & 1
            for mb in range(7):
                votes_sb = vpool.tile([64, 8 * 448], F32, tag="vsb")
                for ic in range(IC):
                    xt_v, po = xtiles[ic]
                    ps = pspool.tile([64, 448], F32, tag="ps")
                    for j in range(4):
                        jh, jw = j >> 1, j & 1
                        dh = DH[ph][jh]
                        dw = DH[pw][jw]
                        r0 = mb * 8 + 1 + dh
                        rhs = xt_v[
                            po : po + 32, r0 : r0 + 8, 1 + dw : 1 + dw + 56
                        ]
                        nc.tensor.matmul(
                            ps[:],
                            wt_sb[
                                po : po + 32,
                                (p * 4 + j) * 64 : (p * 4 + j + 1) * 64,
                            ],
                            rhs,
                            start=(j == 0),
                            stop=(j == 3),
                        )
                    nc.scalar.copy(votes_sb[:, ic * 448 : (ic + 1) * 448], ps[:])

                for q in range(4):
                    tp = tppool.tile([112, 512], F32, tag="tp")
                    for ic in range(IC):
                        nc.tensor.transpose(
                            tp[:, ic * 64 : (ic + 1) * 64],
                            votes_sb[:, ic * 448 + q * 112 : ic * 448 + (q + 1) * 112],
                            ident,
                        )
                    v = pmpool.tile([112, 512], F32, tag="v")
                    nc.scalar.copy(v[:], tp[:])

                    # ---- routing on v [112, (ic,oc,oa)] ----
                    v4 = v[:].rearrange("p (ic oc oa) -> p ic oc oa", ic=8, oc=4)
                    v_jic = v[:].rearrange("p (ic j) -> p j ic", ic=8)

                    # iter 1: r uniform 0.25
                    Sv = rt.tile([112, 64], F32, tag="mid")
                    nc.vector.tensor_reduce(Sv[:], v_jic, axis=AX.X, op=OP.add)
                    t1 = rt.tile([112, 64], F32, tag="mid")
                    nc.vector.scalar_tensor_tensor(
                        t1[:], Sv[:], 0.25, bias_ap, op0=OP.mult, op1=OP.add
                    )
                    act1 = rt.tile([112, 64], F32, tag="actA")
                    _squash_tiles(nc, rt, t1[:], act1[:], "a")

                    dl = rt.tile([112, 32], F32, tag="dlg")
                    act_prev = act1
                    for it in (2, 3):
                        tmp = rt.tile([112, 512], F32, tag="big")
                        a_bc = (
                            act_prev[:]
                            .rearrange("p (oc oa) -> p oc oa", oc=4)
                            .unsqueeze(1)
                            .broadcast_to([112, 8, 4, 16])
                        )
                        tmp4 = tmp[:].rearrange(
                            "p (ic oc oa) -> p ic oc oa", ic=8, oc=4
                        )
                        nc.gpsimd.tensor_mul(tmp4, v4, a_bc)
                        if it == 2:
                            nc.vector.tensor_reduce(
                                dl[:],
                                tmp[:].rearrange("p (g oa) -> p g oa", g=32),
                                axis=AX.X,
                                op=OP.add,
                            )
                        else:
                            dlb = rt.tile([112, 32], F32, tag="mid")
                            nc.vector.tensor_reduce(
                                dlb[:],
                                tmp[:].rearrange("p (g oa) -> p g oa", g=32),
                                axis=AX.X,
                                op=OP.add,
                            )
                            nc.vector.tensor_add(dl[:], dl[:], dlb[:])
                        # softmax over oc (no max-sub; logits are small)
                        e = rt.tile([112, 32], F32, tag="mid")
                        nc.scalar.activation(e[:], dl[:], AF.Exp)
                        se = rt.tile([112, 8], F32, tag="sml")
                        nc.vector.tensor_reduce(
                            se[:],
                            e[:].rearrange("p (ic oc) -> p ic oc", oc=4),
                            axis=AX.X,
                            op=OP.add,
                        )
                        rcp = rt.tile([112, 8], F32, tag="sml")
                        nc.vector.reciprocal(rcp[:], se[:])
                        r = rt.tile([112, 32], F32, tag="mid")
                        nc.vector.tensor_mul(
                            r[:].rearrange("p (ic oc) -> p ic oc", oc=4),
                            e[:].rearrange("p (ic oc) -> p ic oc", oc=4),
                            rcp[:].unsqueeze(2).broadcast_to([112, 8, 4]),
                        )
                        # preact = sum_ic r*v + b
                        rv = rt.tile([112, 512], F32, tag="big")
                        r_bc = (
                            r[:]
                            .rearrange("p (ic oc) -> p ic oc", oc=4)
                            .unsqueeze(3)
                            .broadcast_to([112, 8, 4, 16])
                        )
                        nc.gpsimd.tensor_mul(
                            rv[:].rearrange("p (ic oc oa) -> p ic oc oa", ic=8, oc=4),
                            v4,
                            r_bc,
                        )
                        pre = rt.tile([112, 64], F32, tag="mid")
                        nc.vector.tensor_reduce(
                            pre[:],
                            rv[:].rearrange("p (ic j) -> p j ic", ic=8),
                            axis=AX.X,
                            op=OP.add,
                        )
                        tb = rt.tile([112, 64], F32, tag="mid")
                        nc.vector.tensor_add(tb[:], pre[:], bias_ap)
                        if it == 2:
                            act2 = rt.tile([112, 64], F32, tag="actA")
                            _squash_tiles(nc, rt, tb[:], act2[:], "b")
                            act_prev = act2
                        else:
                            act3 = opool.tile([112, 64], I8, tag="act3")
                            _squash_tiles(
                                nc, rt, tb[:], act3[:], "c", scale=OSCALE
                            )
                            base = mb * 448 + q * 112
                            nc.sync.dma_start(
                                out_d.ap()[p, base : base + 112, :], act3[:]
                            )
    nc.compile()
    _CACHE["nc"] = nc
    return nc


def _get_dispatch():
    if "dispatch" in _CACHE:
        return _CACHE["dispatch"]

    nc = _build_nc()

    import jax
    import jax.numpy as jnp
    from jax.sharding import Mesh, PartitionSpec, NamedSharding
    from jax.experimental.shard_map import shard_map
    from concourse.bass2jax import (
        _bass_exec_p,
        install_neuronx_cc_hook,
        partition_id_tensor,
    )

    try:
        jax.config.update("jax_compilation_cache_dir", "/tmp/jax_cache_caps")
        jax.config.update("jax_persistent_cache_min_compile_time_secs", 0.0)
    except Exception:
        pass

    install_neuronx_cc_hook()

    partition_name = nc.partition_id_tensor.name if nc.partition_id_tensor else None
    in_names, out_names, out_avals = [], [], []
    for alloc in nc.m.functions[0].allocations:
        if not isinstance(alloc, mybir.MemoryLocationSet):
            continue
        name = alloc.memorylocations[0].name
        if alloc.kind == "ExternalInput":
            if name != partition_name:
                in_names.append(name)
        elif alloc.kind == "ExternalOutput":
            shape = tuple(alloc.tensor_shape)
            dtype = mybir.dt.np(alloc.dtype)
            out_avals.append(jax.core.ShapedArray(shape, dtype))
            out_names.append(name)
    n_params = len(in_names)
    n_outs = len(out_avals)
    in_names_full = in_names + out_names + (
        [partition_name] if partition_name else []
    )

    def _body(*args):
        operands = list(args)
        if partition_name is not None:
            operands.append(partition_id_tensor())
        outs = _bass_exec_p.bind(
            *operands,
            out_avals=tuple(out_avals),
            in_names=tuple(in_names_full),
            out_names=tuple(out_names),
            lowering_input_output_aliases=(),
            sim_require_finite=True,
            sim_require_nnan=True,
            nc=nc,
        )
        return tuple(outs)

    n_cores = B
    devices = jax.devices()[:n_cores]
    mesh = Mesh(np.asarray(devices), ("core",))
    in_specs = (PartitionSpec("core"),) * (n_params + n_outs)
    out_specs = (PartitionSpec("core"),) * n_outs
    donate = tuple(range(n_params, n_params + n_outs))
    sharded = jax.jit(
        shard_map(
            _body, mesh=mesh, in_specs=in_specs, out_specs=out_specs,
            check_rep=False,
        ),
        donate_argnums=donate,
        keep_unused=True,
    )
    sh = NamedSharding(mesh, PartitionSpec("core"))
    make_zeros = jax.jit(
        lambda: tuple(
            jnp.zeros((n_cores * av.shape[0], *av.shape[1:]), av.dtype)
            for av in out_avals
        ),
        out_shardings=tuple(sh for _ in out_avals),
    )

    disp = {
        "jax": jax,
        "sharded": sharded,
        "make_zeros": make_zeros,
        "in_names": in_names,
        "out_names": out_names,
        "out_avals": out_avals,
        "sh": sh,
        "n_cores": n_cores,
    }
    _CACHE["dispatch"] = disp
    return disp


def _prep_x(input_tensor):
    """Concatenated (8*3136, 256) fp16 pixel-major input, reusing the buffer."""
    x = np.asarray(input_tensor, np.float32)
    xall = _CACHE.get("xbuf")
    if xall is None:
        xall = np.zeros((B, NPIX, IC * IA), np.float16)
        _CACHE["xbuf"] = xall
    xall[:] = x.reshape(B, NPIX, IC * IA)
    return xall.reshape(B * NPIX, IC * IA)


def _prep_static(W, b):
    Wk = np.asarray(W, np.float32)
    bb = np.asarray(b, np.float32).reshape(OC, OA)
    wt = np.zeros((32, 1024), np.float16)
    for p in range(4):
        ph, pw = p >> 1, p & 1
        for j in range(4):
            jh, jw = j >> 1, j & 1
            kh, kw = KH[ph][jh], KH[pw][jw]
            wt[:, (p * 4 + j) * 64 : (p * 4 + j + 1) * 64] = Wk[kh, kw].T
    wtall = np.broadcast_to(np.tile(wt, (4, 1))[None], (B, 128, 1024))
    cst = np.zeros((128, 128), np.float32)
    cst[:, :64] = bb.reshape(1, 64)
    cst[:64, 64:128] = np.eye(64, dtype=np.float32)
    cstall = np.broadcast_to(cst[None], (B, 128, 128))
    return {
        "wt": np.ascontiguousarray(wtall).reshape(B * 128, 1024),
        "cst": np.ascontiguousarray(cstall).reshape(B * 128, 128),
    }


def _prep_inputs(input_tensor, W, b):
    """Back-compat helper for profiling scripts."""
    out = {"x": _prep_x(input_tensor)}
    out.update(_prep_static(W, b))
    return out


def _unshard(out_concat):
    # out_concat: [(B*4), NPIX, 64] int8, fixed-point scale OSCALE
    from concurrent.futures import ThreadPoolExecutor

    pool = _CACHE.get("tpool")
    if pool is None:
        pool = ThreadPoolExecutor(8)
        _CACHE["tpool"] = pool
    o8 = np.asarray(out_concat).reshape(B, 2, 2, 56, 56, OC, OA)
    res = np.empty((B, 112, 112, OC, OA), np.float32)

    def one(bi):
        t = o8[bi].astype(np.float32)
        t *= np.float32(1.0 / OSCALE)
        # (ph, pw, 56, 56, OC, OA) -> (56, ph, 56, pw, OC, OA)
        res[bi] = t.transpose(2, 0, 3, 1, 4, 5).reshape(112, 112, OC, OA)

    list(pool.map(one, range(B)))
    return res


def kernel(input_tensor, W, b):
    disp = _get_dispatch()
    jax = disp["jax"]

    # issue the x upload first so the transfer runs while the host does the
    # static-weight check; wt/cst device copies are re-made only when W or b
    # actually change
    dev_x = jax.device_put(_prep_x(input_tensor), disp["sh"])
    wkey = (
        np.asarray(W, np.float32).tobytes(),
        np.asarray(b, np.float32).tobytes(),
    )
    dev_static = _CACHE.get("dev_static")
    if dev_static is None or dev_static[0] != wkey:
        stat = _prep_static(W, b)
        dev_w = {
            name: jax.device_put(stat[name], disp["sh"])
            for name in disp["in_names"]
            if name != "x"
        }
        dev_static = (wkey, dev_w)
        _CACHE["dev_static"] = dev_static
    dev_in = [
        dev_x if name == "x" else dev_static[1][name]
        for name in disp["in_names"]
    ]

    # donated output buffers: recycle the previous call's outputs (the kernel
    # fully overwrites them); only the first call pays the make_zeros dispatch
    donate_bufs = _CACHE.pop("donate_bufs", None)
    if donate_bufs is None:
        donate_bufs = disp["make_zeros"]()
    out_arrs = disp["sharded"](*dev_in, *donate_bufs)
    try:
        out_arrs[0].copy_to_host_async()
    except Exception:
        pass
    out_concat = np.asarray(out_arrs[0])
    _CACHE["donate_bufs"] = out_arrs
    return _unshard(out_concat)


# revision 6
# speedup vs baseline: 1.0740x; 1.0740x over previous
"""DeconvCapsuleLayer Trainium2 kernel, v2.

Data-parallel over batch (B=8 -> 1 image per NeuronCore).
Device kernel per core (same structure as v1, fp16 I/O):
  - deconv as 4 sub-pixel phases; each phase = 4 taps of K=32 fp16 matmuls
    accumulated in fp32 PSUM (W stationary, out = [64(oc,oa), pixels]).
  - PE transpose to pixel-major [pixels, (ic,oc,oa)].
  - dynamic routing (3 iters) on DVE/ACT in fp32, output stored fp16.
Host: fp16 inputs/outputs to halve the axon link traffic; the jitted
shard_map dispatch is built once and cached (the stock
run_bass_kernel_spmd path re-traces and re-lowers on every call).
"""

import os
import sys
from contextlib import ExitStack

import numpy as np

for _p in ("/opt/trn_rl_repo", os.path.expanduser("~/.axon_site/_ro/trn_rl_repo")):
    if os.path.isdir(_p) and _p not in sys.path:
        sys.path.insert(0, _p)

import concourse.bass as bass
import concourse.bacc as bacc
import concourse.tile as tile
from concourse import mybir
from concourse.bass_utils import run_bass_kernel_spmd  # noqa: F401 (fallback)

F32 = mybir.dt.float32
F16 = mybir.dt.float16
I8 = mybir.dt.int8
OSCALE = 124.0  # int8 output fixed-point scale (|act| < 1; margin for rounding)
AX = mybir.AxisListType
OP = mybir.AluOpType
AF = mybir.ActivationFunctionType

B, H, Wd, IC, IA = 8, 56, 56, 8, 32
OC, OA = 4, 16
PH, PW = 58, 58  # padded input spatial
NPIX = 56 * 56   # pixels per phase image
# tap tables: KH[parity] = kernel taps, DH[parity] = input shifts
KH = {0: [1, 3], 1: [0, 2]}
DH = {0: [0, -1], 1: [1, 0]}

_CACHE = {}


def _squash_tiles(nc, pool, t_ap, out_ap, tag, scale=None):
    """out = t * sqrt(nsq)/(1+nsq) [* scale], nsq = sum_oa t^2  (t: [112, 64])."""
    sq = pool.tile([112, 64], F32, tag="mid")
    nc.vector.tensor_mul(sq[:], t_ap, t_ap)
    nsq = pool.tile([112, 4], F32, tag="sml")
    nc.vector.tensor_reduce(
        nsq[:], sq[:].rearrange("p (oc oa) -> p oc oa", oc=4), axis=AX.X, op=OP.add
    )
    s = pool.tile([112, 4], F32, tag="sml")
    nc.scalar.sqrt(s[:], nsq[:])
    u = pool.tile([112, 4], F32, tag="sml")
    nc.vector.tensor_scalar_add(u[:], nsq[:], 1.0)
    rc = pool.tile([112, 4], F32, tag="sml")
    nc.vector.reciprocal(rc[:], u[:])
    f = pool.tile([112, 4], F32, tag="sml")
    if scale is None:
        nc.vector.tensor_mul(f[:], s[:], rc[:])
    else:
        nc.vector.scalar_tensor_tensor(
            f[:], s[:], float(scale), rc[:], op0=OP.mult, op1=OP.mult
        )
    f_bc = f[:].unsqueeze(2).broadcast_to([112, 4, 16])
    t3 = t_ap.rearrange("p (oc oa) -> p oc oa", oc=4)
    nc.vector.tensor_mul(out_ap.rearrange("p (oc oa) -> p oc oa", oc=4), t3, f_bc)


def _build_nc():
    if "nc" in _CACHE:
        return _CACHE["nc"]
    nc = bacc.Bacc("TRN2", target_bir_lowering=False, debug=False)
    # raw pixel-major input [h*w, ic*ia]; transposed to capsule-major on device
    x_d = nc.dram_tensor("x", [NPIX, IC * IA], F16, kind="ExternalInput")
    wt_d = nc.dram_tensor("wt", [128, 1024], F16, kind="ExternalInput")
    cst_d = nc.dram_tensor("cst", [128, 128], F32, kind="ExternalInput")
    out_d = nc.dram_tensor("out", [4, NPIX, 64], I8, kind="ExternalOutput")

    with tile.TileContext(nc) as tc, ExitStack() as ctx:
        cpool = ctx.enter_context(tc.tile_pool(name="const", bufs=1))
        xwpool = ctx.enter_context(tc.tile_pool(name="xw", bufs=2))
        wt_sb = cpool.tile([128, 1024], F16, tag="wt")
        nc.sync.dma_start(wt_sb[:], wt_d.ap())
        cst_sb = cpool.tile([128, 128], F32, tag="cst")
        nc.sync.dma_start(cst_sb[:], cst_d.ap())
        bias_ap = cst_sb[0:112, 0:64]
        ident = cst_sb[0:64, 64:128]

        vpool = ctx.enter_context(tc.tile_pool(name="votes", bufs=2))
        pmpool = ctx.enter_context(tc.tile_pool(name="pm", bufs=2))
        pspool = ctx.enter_context(tc.tile_pool(name="ps", bufs=2, space="PSUM"))
        tppool = ctx.enter_context(tc.tile_pool(name="tp", bufs=2, space="PSUM"))
        rt = ctx.enter_context(tc.tile_pool(name="rt", bufs=10))
        opool = ctx.enter_context(tc.tile_pool(name="outp", bufs=3))

        # On-device layout transform: XBAR DMA transpose [3136, 256] ->
        # two [128, 3136] halves written into zero-padded [128, 58, 58]
        # capsule-major images (partitions = (ic_local, ia), ic-major).
        # matmul operands must share base partition in {0, 32, 64}; the 4th
        # ic block (partitions 96:128) is relocated to its own base-0 tile.
        xtiles = {}
        for hi in range(2):
            xt = cpool.tile([128, PH * PW], F16, tag=f"xsb{hi}")
            xt_v = xt[:].rearrange("k (h w) -> k h w", h=PH, w=PW)
            nc.vector.memset(xt_v[:, 0:1, :], 0.0)
            nc.vector.memset(xt_v[:, 57:58, :], 0.0)
            nc.vector.memset(xt_v[:, 1:57, 0:1], 0.0)
            nc.vector.memset(xt_v[:, 1:57, 57:58], 0.0)
            xraw = cpool.tile([128, NPIX], F16, tag=f"xraw{hi}")
            nc.sync.dma_start_transpose(
                xraw[:], x_d.ap()[:, hi * 128 : (hi + 1) * 128]
            )
            nc.scalar.copy(
                xt_v[:, 1:57, 1:57],
                xraw[:].rearrange("k (h w) -> k h w", h=56, w=56),
            )
            xc = cpool.tile([32, PH * PW], F16, tag=f"xc{hi}")
            nc.sync.dma_start(xc[:], xt[96:128, :])
            xc_v = xc[:].rearrange("k (h w) -> k h w", h=PH, w=PW)
            for loc in range(3):
                xtiles[hi * 4 + loc] = (xt_v, loc * 32)
            xtiles[hi * 4 + 3] = (xc_v, 0)

        for p in range(4):
            ph, pw = p >> 1, p & 1
            for mb in range(7):
                votes_sb = vpool.tile([64, 8 * 448], F32, tag="vsb")
                for ic in range(IC):
                    xt_v, po = xtiles[ic]
                    ps = pspool.tile([64, 448], F32, tag="ps")
                    for j in range(4):
                        jh, jw = j >> 1, j & 1
                        dh = DH[ph][jh]
                        dw = DH[pw][jw]
                        r0 = mb * 8 + 1 + dh
                        rhs = xt_v[
                            po : po + 32, r0 : r0 + 8, 1 + dw : 1 + dw + 56
                        ]
                        nc.tensor.matmul(
                            ps[:],
                            wt_sb[
                                po : po + 32,
                                (p * 4 + j) * 64 : (p * 4 + j + 1) * 64,
                            ],
                            rhs,
                            start=(j == 0),
                            stop=(j == 3),
                        )
                    nc.scalar.copy(votes_sb[:, ic * 448 : (ic + 1) * 448], ps[:])

                for q in range(4):
                    tp = tppool.tile([112, 512], F32, tag="tp")
                    for ic in range(IC):
                        nc.tensor.transpose(
                            tp[:, ic * 64 : (ic + 1) * 64],
                            votes_sb[:, ic * 448 + q * 112 : ic * 448 + (q + 1) * 112],
                            ident,
                        )
                    v = pmpool.tile([112, 512], F32, tag="v")
                    nc.scalar.copy(v[:], tp[:])

                    # ---- routing on v [112, (ic,oc,oa)] ----
                    v4 = v[:].rearrange("p (ic oc oa) -> p ic oc oa", ic=8, oc=4)
                    v_jic = v[:].rearrange("p (ic j) -> p j ic", ic=8)

                    # iter 1: r uniform 0.25
                    Sv = rt.tile([112, 64], F32, tag="mid")
                    nc.vector.tensor_reduce(Sv[:], v_jic, axis=AX.X, op=OP.add)
                    t1 = rt.tile([112, 64], F32, tag="mid")
                    nc.vector.scalar_tensor_tensor(
                        t1[:], Sv[:], 0.25, bias_ap, op0=OP.mult, op1=OP.add
                    )
                    act1 = rt.tile([112, 64], F32, tag="actA")
                    _squash_tiles(nc, rt, t1[:], act1[:], "a")

                    dl = rt.tile([112, 32], F32, tag="dlg")
                    act_prev = act1
                    for it in (2, 3):
                        tmp = rt.tile([112, 512], F32, tag="big")
                        a_bc = (
                            act_prev[:]
                            .rearrange("p (oc oa) -> p oc oa", oc=4)
                            .unsqueeze(1)
                            .broadcast_to([112, 8, 4, 16])
                        )
                        tmp4 = tmp[:].rearrange(
                            "p (ic oc oa) -> p ic oc oa", ic=8, oc=4
                        )
                        nc.gpsimd.tensor_mul(tmp4, v4, a_bc)
                        if it == 2:
                            nc.vector.tensor_reduce(
                                dl[:],
                                tmp[:].rearrange("p (g oa) -> p g oa", g=32),
                                axis=AX.X,
                                op=OP.add,
                            )
                        else:
                            dlb = rt.tile([112, 32], F32, tag="mid")
                            nc.vector.tensor_reduce(
                                dlb[:],
                                tmp[:].rearrange("p (g oa) -> p g oa", g=32),
                                axis=AX.X,
                                op=OP.add,
                            )
                            nc.vector.tensor_add(dl[:], dl[:], dlb[:])
                        # softmax over oc (no max-sub; logits are small)
                        e = rt.tile([112, 32], F32, tag="mid")
                        nc.scalar.activation(e[:], dl[:], AF.Exp)
                        se = rt.tile([112, 8], F32, tag="sml")
                        nc.vector.tensor_reduce(
                            se[:],
                            e[:].rearrange("p (ic oc) -> p ic oc", oc=4),
                            axis=AX.X,
                            op=OP.add,
                        )
                        rcp = rt.tile([112, 8], F32, tag="sml")
                        nc.vector.reciprocal(rcp[:], se[:])
                        r = rt.tile([112, 32], F32, tag="mid")
                        nc.vector.tensor_mul(
                            r[:].rearrange("p (ic oc) -> p ic oc", oc=4),
                            e[:].rearrange("p (ic oc) -> p ic oc", oc=4),
                            rcp[:].unsqueeze(2).broadcast_to([112, 8, 4]),
                        )
                        # preact = sum_ic r*v + b
                        rv = rt.tile([112, 512], F32, tag="big")
                        r_bc = (
                            r[:]
                            .rearrange("p (ic oc) -> p ic oc", oc=4)
                            .unsqueeze(3)
                            .broadcast_to([112, 8, 4, 16])
                        )
                        nc.gpsimd.tensor_mul(
                            rv[:].rearrange("p (ic oc oa) -> p ic oc oa", ic=8, oc=4),
                            v4,
                            r_bc,
                        )
                        pre = rt.tile([112, 64], F32, tag="mid")
                        nc.vector.tensor_reduce(
                            pre[:],
                            rv[:].rearrange("p (ic j) -> p j ic", ic=8),
                            axis=AX.X,
                            op=OP.add,
                        )
                        tb = rt.tile([112, 64], F32, tag="mid")
                        nc.vector.tensor_add(tb[:], pre[:], bias_ap)
                        if it == 2:
                            act2 = rt.tile([112, 64], F32, tag="actA")
                            _squash_tiles(nc, rt, tb[:], act2[:], "b")
                            act_prev = act2
                        else:
                            act3 = opool.tile([112, 64], I8, tag="act3")
                            _squash_tiles(
                                nc, rt, tb[:], act3[:], "c", scale=OSCALE
                            )
                            base = mb * 448 + q * 112
                            nc.sync.dma_start(
                                out_d.ap()[p, base : base + 112, :], act3[:]
                            )
    nc.compile()
    _CACHE["nc"] = nc
    return nc


def _get_dispatch():
    if "dispatch" in _CACHE:
        return _CACHE["dispatch"]

    nc = _build_nc()

    import jax
    import jax.numpy as jnp
    from jax.sharding import Mesh, PartitionSpec, NamedSharding
    from jax.experimental.shard_map import shard_map
    from concourse.bass2jax import (
        _bass_exec_p,
        install_neuronx_cc_hook,
        partition_id_tensor,
    )

    try:
        jax.config.update("jax_compilation_cache_dir", "/tmp/jax_cache_caps")
        jax.config.update("jax_persistent_cache_min_compile_time_secs", 0.0)
    except Exception:
        pass

    install_neuronx_cc_hook()

    partition_name = nc.partition_id_tensor.name if nc.partition_id_tensor else None
    in_names, out_names, out_avals = [], [], []
    for alloc in nc.m.functions[0].allocations:
        if not isinstance(alloc, mybir.MemoryLocationSet):
            continue
        name = alloc.memorylocations[0].name
        if alloc.kind == "ExternalInput":
            if name != partition_name:
                in_names.append(name)
        elif alloc.kind == "ExternalOutput":
            shape = tuple(alloc.tensor_shape)
            dtype = mybir.dt.np(alloc.dtype)
            out_avals.append(jax.core.ShapedArray(shape, dtype))
            out_names.append(name)
    n_params = len(in_names)
    n_outs = len(out_avals)
    in_names_full = in_names + out_names + (
        [partition_name] if partition_name else []
    )

    def _body(*args):
        operands = list(args)
        if partition_name is not None:
            operands.append(partition_id_tensor())
        outs = _bass_exec_p.bind(
            *operands,
            out_avals=tuple(out_avals),
            in_names=tuple(in_names_full),
            out_names=tuple(out_names),
            lowering_input_output_aliases=(),
            sim_require_finite=True,
            sim_require_nnan=True,
            nc=nc,
        )
        return tuple(outs)

    n_cores = B
    devices = jax.devices()[:n_cores]
    mesh = Mesh(np.asarray(devices), ("core",))
    in_specs = (PartitionSpec("core"),) * (n_params + n_outs)
    out_specs = (PartitionSpec("core"),) * n_outs
    donate = tuple(range(n_params, n_params + n_outs))
    sharded = jax.jit(
        shard_map(
            _body, mesh=mesh, in_specs=in_specs, out_specs=out_specs,
            check_rep=False,
        ),
        donate_argnums=donate,
        keep_unused=True,
    )
    sh = NamedSharding(mesh, PartitionSpec("core"))
    make_zeros = jax.jit(
        lambda: tuple(
            jnp.zeros((n_cores * av.shape[0], *av.shape[1:]), av.dtype)
            for av in out_avals
        ),
        out_shardings=tuple(sh for _ in out_avals),
    )

    disp = {
        "jax": jax,
        "sharded": sharded,
        "make_zeros": make_zeros,
        "in_names": in_names,
        "out_names": out_names,
        "out_avals": out_avals,
        "sh": sh,
        "n_cores": n_cores,
    }
    _CACHE["dispatch"] = disp
    return disp


def _prep_x(input_tensor):
    """Concatenated (8*3136, 256) fp16 pixel-major input, reusing the buffer."""
    x = np.asarray(input_tensor, np.float32)
    xall = _CACHE.get("xbuf")
    if xall is None:
        xall = np.zeros((B, NPIX, IC * IA), np.float16)
        _CACHE["xbuf"] = xall
    xall[:] = x.reshape(B, NPIX, IC * IA)
    return xall.reshape(B * NPIX, IC * IA)


def _prep_static(W, b):
    Wk = np.asarray(W, np.float32)
    bb = np.asarray(b, np.float32).reshape(OC, OA)
    wt = np.zeros((32, 1024), np.float16)
    for p in range(4):
        ph, pw = p >> 1, p & 1
        for j in range(4):
            jh, jw = j >> 1, j & 1
            kh, kw = KH[ph][jh], KH[pw][jw]
            wt[:, (p * 4 + j) * 64 : (p * 4 + j + 1) * 64] = Wk[kh, kw].T
    wtall = np.broadcast_to(np.tile(wt, (4, 1))[None], (B, 128, 1024))
    cst = np.zeros((128, 128), np.float32)
    cst[:, :64] = bb.reshape(1, 64)
    cst[:64, 64:128] = np.eye(64, dtype=np.float32)
    cstall = np.broadcast_to(cst[None], (B, 128, 128))
    return {
        "wt": np.ascontiguousarray(wtall).reshape(B * 128, 1024),
        "cst": np.ascontiguousarray(cstall).reshape(B * 128, 128),
    }


def _prep_inputs(input_tensor, W, b):
    """Back-compat helper for profiling scripts."""
    out = {"x": _prep_x(input_tensor)}
    out.update(_prep_static(W, b))
    return out


def _unshard(out_concat):
    # out_concat: [(B*4), NPIX, 64] int8, fixed-point scale OSCALE
    from concurrent.futures import ThreadPoolExecutor

    pool = _CACHE.get("tpool")
    if pool is None:
        pool = ThreadPoolExecutor(8)
        _CACHE["tpool"] = pool
    o8 = np.asarray(out_concat).reshape(B, 2, 2, 56, 56, OC, OA)
    res = np.empty((B, 112, 112, OC, OA), np.float32)

    def one(bi):
        t = o8[bi].astype(np.float32)
        t *= np.float32(1.0 / OSCALE)
        # (ph, pw, 56, 56, OC, OA) -> (56, ph, 56, pw, OC, OA)
        res[bi] = t.transpose(2, 0, 3, 1, 4, 5).reshape(112, 112, OC, OA)

    list(pool.map(one, range(B)))
    return res


def kernel(input_tensor, W, b):
    disp = _get_dispatch()
    jax = disp["jax"]

    # issue the x upload first so the transfer runs while the host does the
    # static-weight check; wt/cst device copies are re-made only when W or b
    # actually change
    dev_x = jax.device_put(_prep_x(input_tensor), disp["sh"])
    wkey = (
        np.asarray(W, np.float32).tobytes(),
        np.asarray(b, np.float32).tobytes(),
    )
    dev_static = _CACHE.get("dev_static")
    if dev_static is None or dev_static[0] != wkey:
        stat = _prep_static(W, b)
        dev_w = {
            name: jax.device_put(stat[name], disp["sh"])
            for name in disp["in_names"]
            if name != "x"
        }
        dev_static = (wkey, dev_w)
        _CACHE["dev_static"] = dev_static
    dev_in = [
        dev_x if name == "x" else dev_static[1][name]
        for name in disp["in_names"]
    ]

    # donated output buffers: recycle the previous call's outputs (the kernel
    # fully overwrites them); only the first call pays the make_zeros dispatch
    donate_bufs = _CACHE.pop("donate_bufs", None)
    if donate_bufs is None:
        donate_bufs = disp["make_zeros"]()
    out_arrs = disp["sharded"](*dev_in, *donate_bufs)
    try:
        out_arrs[0].copy_to_host_async()
    except Exception:
        pass
    out_concat = np.asarray(out_arrs[0])
    _CACHE["donate_bufs"] = out_arrs
    return _unshard(out_concat)
